# revision 11
# baseline (speedup 1.0000x reference)
"""Trainium2 Bass kernel for nn_CrossAttentionBlock (B=8, N=1024, C=768, H=12).

Sharding: data-parallel over the batch dim - each of the 8 NeuronCores runs the
full cross-attention block for one batch element. No collectives.

v2 vs v1: the Activation engine (softmax exp) was the bottleneck at 85% busy.
Rebalanced by computing roughly half the exp tiles on DVE via a Schraudolph
bit-trick (uint8(A*S + B) bitcast to fp8e4m3 approximates exp(S/8) to ~3% RMS,
which is at the same level as the fp8 quantization the E tiles already get),
moving all PSUM evacuations to whichever of ACT/DVE balances, widening the
Q/K projection blocks to 128 partitions (4 heads x 32, tile_position=(96,0)
for the 4th head), folding the V bias into a bf16 rank-1 PE matmul so the V
evacuation is a pure copy, batching the softmax-denominator reciprocals in
head pairs via tiny SBUF->SBUF collector DMAs, and computing the LayerNorm
sum-of-squares on DVE (scalar_tensor_tensor with accum_out).
"""

import json

import ml_dtypes
import numpy as np

import concourse.bass as bass
import concourse.mybir as mybir
import concourse.tile as tile

B, N, C, H, D = 8, 1024, 768, 12, 64
KB = C // 128  # feature-dim 128-blocks
TB = N // 128  # token-dim 128-blocks
NCH = 3        # head chunks of 4 heads (partition bases 0/32/64/96)
NJ = 2 * NCH   # 128-wide packed Q/K output blocks per tensor
CP = NJ * 128  # packed Q/K out-feature count (768)
SCALE = D ** -0.5
EPS = 1e-5
F32 = mybir.dt.float32
BF16 = mybir.dt.bfloat16
FP8 = mybir.dt.float8e4
U8 = mybir.dt.uint8
AF = mybir.ActivationFunctionType
ALU = mybir.AluOpType
DR = mybir.MatmulPerfMode.DoubleRow
F8_NP = ml_dtypes.float8_e4m3

# Schraudolph exp: uint8(A*x + B) bitcast fp8e4m3 ~= exp(x), RMS err ~3.1%.
# B tuned for zero mean relative error so mixed exact/trick softmax rows
# stay consistent.  Range check: S*SCALE in [-2.9, 2.9] -> byte in [22, 89].
A_TRICK = 8.0 / np.log(2.0)
B_TRICK = 56.0 - 0.45

# ---------------------------------------------------------------------------
# Workaround: this walrus build rejects instructions with more than one
# semaphore wait ("Too many sync wait commands").  Legalize the BIR by hoisting
# excess waits onto same-engine NoOps inserted right before the instruction.
# ---------------------------------------------------------------------------
_MAX_WAITS = 1
_legal_counter = [0]


def _legalize_waits(bir_json: bytes) -> bytes:
    m = json.loads(bir_json)
    changed = False
    for fn in m.get("functions", []):
        for bb in fn.get("blocks", []):
            out = []
            for inst in bb.get("instructions", []):
                si = inst.get("sync_info") or {}
                waits = si.get("on_wait") or []
                if len(waits) > _MAX_WAITS:
                    changed = True
                    extra = waits[_MAX_WAITS:]
                    si["on_wait"] = waits[:_MAX_WAITS]
                    for i in range(0, len(extra), _MAX_WAITS):
                        _legal_counter[0] += 1
                        nop = {
                            "engine": inst["engine"],
                            "ins": [],
                            "name": f"I-legalw-{_legal_counter[0]}",
                            "opcode": "NoOp",
                            "outs": [],
                            "sync_info": {
                                "on_update": [],
                                "on_wait": extra[i : i + _MAX_WAITS],
                            },
                        }
                        if "debug" in inst:
                            nop["debug"] = inst["debug"]
                        out.append(nop)
                out.append(inst)
            bb["instructions"] = out
    return json.dumps(m).encode() if changed else bir_json


_hooked = False


def _install_compile_hook():
    global _hooked
    if _hooked:
        return
    _hooked = True
    import concourse.bass_utils as bu

    orig = bu.compile_bir_kernel

    def compile_bir_kernel(bir_json, tmpdir, neff_name="file.neff"):
        return orig(_legalize_waits(bir_json), tmpdir, neff_name)

    bu.compile_bir_kernel = compile_bir_kernel
    try:
        import concourse.bass2jax as b2j

        b2j.compile_bir_kernel = compile_bir_kernel
    except ImportError:
        pass


# ---------------------------------------------------------------------------
# Kernel builder
# ---------------------------------------------------------------------------

def _dram_ap(t, offset, ap):
    return bass.AP(t, offset, ap)


def build_nc() -> bass.Bass:
    nc = bass.Bass()

    QW_d = nc.dram_tensor("QW8", [C, CP + N], FP8, kind="ExternalInput")
    KW_d = nc.dram_tensor("KW8", [C, CP + N], FP8, kind="ExternalInput")
    WvT_d = nc.dram_tensor("WvT8", [C, C], FP8, kind="ExternalInput")
    WoT_d = nc.dram_tensor("WoT8", [C, C + 1], FP8, kind="ExternalInput")
    bqk = nc.dram_tensor("bqk", [128, 2 * NJ], F32, kind="ExternalInput")
    bvT = nc.dram_tensor("bvT", [1, C], BF16, kind="ExternalInput")
    qres_d = nc.dram_tensor("q_res", [N, C + 1], BF16, kind="ExternalInput")
    ident_d = nc.dram_tensor("ident", [128, 128], BF16, kind="ExternalInput")
    out_t = nc.dram_tensor("out", [N, C], BF16, kind="ExternalOutput")
    rsc_d = nc.dram_tensor("r_scratch", [H, N], F32, kind="Internal")

    with tile.TileContext(nc) as tc:
        _body(tc, nc, (QW_d, KW_d, WvT_d, WoT_d),
              (bqk, bvT), qres_d, ident_d, out_t, rsc_d)
    return nc


def _body(tc, nc, WTs, bs, qres_d, ident_d, out_t, rsc_d):
    QW_d, KW_d, WvT_d, WoT_d = WTs
    bqk, bvT = bs

    with (
        tc.tile_pool(name="singles", bufs=1) as singles,
        tc.tile_pool(name="feat", bufs=1) as feat,
        tc.tile_pool(name="attn", bufs=1) as attn,
        tc.tile_pool(name="epi", bufs=1) as epi,
        tc.tile_pool(name="ps", bufs=1, space="PSUM") as ps,
        tc.tile_pool(name="psP", bufs=1, space="PSUM") as psP,
        tc.tile_pool(name="psO", bufs=1, space="PSUM") as psO,
    ):
        # ---- constants / biases (tiny, issued first) --------------------
        bqk_sb = singles.tile([128, 2, NJ], F32, name="bqk_sb")
        nc.sync.dma_start(
            out=bqk_sb, in_=_dram_ap(bqk, 0, [[2 * NJ, 128], [1, 2 * NJ]])
        )
        bq_sb = bqk_sb[:, 0, :]
        bk_sb = bqk_sb[:, 1, :]
        eps_t = singles.tile([128, 1], F32, name="eps_t")
        nc.vector.memset(eps_t, EPS)
        warm = singles.tile([1, 512], FP8, name="warm")
        nc.vector.memset(warm, 0.0)
        ones_bf = singles.tile([1, 128], BF16, name="ones_bf")
        nc.vector.memset(ones_bf, 1.0)
        bvT_sb = singles.tile([1, C], BF16, name="bvT_sb")
        nc.sync.dma_start(out=bvT_sb, in_=_dram_ap(bvT, 0, [[C, 1], [1, C]]))
        pwarm = psP.tile([128, N], F32, name="pwarm", tag="P", bufs=1)
        for _ in range(8):
            nc.tensor.matmul(
                pwarm[0:1, 0:512], warm[0:1, 0:1], warm, start=True, stop=True
            )

        # ---- long-lived fp8 feature-major tensors -----------------------
        CPN = CP + N
        QW = feat.tile([128, KB, CPN], FP8, name="QW")
        KW = feat.tile([128, KB, CPN], FP8, name="KW")
        for t_sb, t_d in ((QW, QW_d), (KW, KW_d)):
            for k0, k1 in ((0, 4), (4, KB)):
                nc.sync.dma_start(
                    out=t_sb[:, k0:k1, :],
                    in_=_dram_ap(
                        t_d, k0 * 128 * CPN,
                        [[CPN, 128], [128 * CPN, k1 - k0], [1, CPN]],
                    ),
                )
        WqT = QW[:, :, 0:CP]
        qT = QW[:, :, CP:CPN]
        WkT = KW[:, :, 0:CP]
        cT = KW[:, :, CP:CPN]
        WvT = feat.tile([128, KB, C], FP8, name="WvT")
        nc.sync.dma_start(
            out=WvT, in_=_dram_ap(WvT_d, 0, [[C, 128], [128 * C, KB], [1, C]])
        )
        CO = C + 1
        WoT = feat.tile([128, KB, CO], FP8, name="WoT")
        nc.sync.dma_start(
            out=WoT, in_=_dram_ap(WoT_d, 0, [[CO, 128], [128 * CO, KB], [1, CO]])
        )
        CQ = C + 1
        q_sb = feat.tile([128, TB, CQ], BF16, name="q_sb")
        nc.sync.dma_start(
            out=q_sb, in_=_dram_ap(qres_d, 0, [[CQ, 128], [128 * CQ, TB], [1, CQ]])
        )
        ident = feat.tile([128, 128], BF16, name="ident")
        nc.sync.dma_start(out=ident, in_=_dram_ap(ident_d, 0, [[128, 128], [1, 128]]))

        # Q/K packed layout [128, chunk, sub, N]: block j=2c+s holds, at
        # partition p, head 4c+p//32, feature d=(p%32)+32s.
        QTs = feat.tile([128, NCH, 2, N], FP8, name="QTs")
        KTs = feat.tile([128, NCH, 2, N], FP8, name="KTs")
        # V token-major, one 128-wide slot per head: cols 0-63 = V, col 64 =
        # ones (softmax row sums ride along in the O matmul), 65+ = zeros.
        V2 = feat.tile([128, TB, H, 128], FP8, name="V2")
        nc.gpsimd.memset(V2[:, :, :, D : D + 1], 1.0)
        nc.gpsimd.memset(V2[:, :, :, D + 1 : 128], 0.0)
        AO = feat.tile([128, KB, N], FP8, name="AO")

        def proj_qk(c, first):
            """Q/K projection blocks j=2c,2c+1 (128 partitions = 4 heads),
            bias-add fused into the fp8 evacuation."""
            for ti, (wT, srcT, b_sb, dstT) in enumerate((
                (WqT, qT, bq_sb, QTs),
                (WkT, cT, bk_sb, KTs),
            )):
                for s in range(2):
                    j = 2 * c + s
                    if first:
                        pj = ps.tile([128, N], F32, name="pj", tag="sq", bufs=2)
                    else:
                        pj = psP.tile([128, N], F32, name="pj", tag="P", bufs=1)
                    for kbp in range(KB // 2):
                        lhsT = wT[:, 2 * kbp : 2 * kbp + 2,
                                  j * 128 : (j + 1) * 128]
                        for ch in range(2):
                            nc.tensor.matmul(
                                pj[:, ch * 512 : (ch + 1) * 512],
                                lhsT,
                                srcT[:, 2 * kbp : 2 * kbp + 2,
                                     ch * 512 : (ch + 1) * 512],
                                start=(kbp == 0),
                                stop=(kbp == KB // 2 - 1),
                                perf_mode=DR,
                            )
                    # evacuation engine: alternate ACT/DVE so both start
                    # early during chunk 0; prefetched chunks go to ACT.
                    on_act = first and (ti == 0) or (not first)
                    splits = (0, 256, N) if (first and ti == 1 and s == 0) \
                        else (0, N)
                    for c0, c1 in zip(splits, splits[1:]):
                        if on_act:
                            nc.scalar.activation(
                                out=dstT[:, c, s, c0:c1],
                                in_=pj[:, c0:c1],
                                func=AF.Identity,
                                bias=b_sb[:, j : j + 1], scale=1.0,
                            )
                        else:
                            nc.vector.tensor_scalar(
                                out=dstT[:, c, s, c0:c1],
                                in0=pj[:, c0:c1],
                                scalar1=b_sb[:, j : j + 1], scalar2=None,
                                op0=ALU.add,
                            )

        def proj_v():
            for tb in range(TB):
                pvt = psP.tile([128, N], F32, name="pv", tag="P", bufs=1)
                pv = pvt[:, 0:C]
                for kbp in range(KB // 2):
                    lhsT = cT[:, 2 * kbp : 2 * kbp + 2, tb * 128 : (tb + 1) * 128]
                    for c0, c1 in ((0, 512), (512, C)):
                        nc.tensor.matmul(
                            pv[:, c0:c1], lhsT,
                            WvT[:, 2 * kbp : 2 * kbp + 2, c0:c1],
                            start=(kbp == 0), stop=False,
                            perf_mode=DR,
                        )
                # bias: rank-1 bf16 matmul ones^T x bvT accumulated into the
                # same PSUM group, so the evacuation is a pure quantize copy
                # (engine-flexible).
                for c0, c1 in ((0, 512), (512, C)):
                    nc.tensor.matmul(
                        pv[:, c0:c1],
                        ones_bf[0:1, 0:128],
                        bvT_sb[0:1, c0:c1],
                        start=False, stop=True,
                    )
                nc.scalar.copy(
                    out=V2[:, tb, :, 0:D],
                    in_=pv.rearrange("p (h d) -> p h d", h=H),
                )

        def attend(h, first):
            c, hp = h // 4, h % 4
            p0 = hp * 32
            kbh, ro = h // 2, D * (h % 2)
            # All 8 S matmuls (and their exps) are issued BEFORE the O
            # matmuls (program order on PE keeps the S stream fed).
            O = psO.tile([128, N], F32, name="O", tag="O", bufs=1)
            E2s = []
            for t in range(TB // 2):
                E2 = attn.tile([128, 2, N], FP8, name="E2", tag="E2", bufs=8)
                E2s.append(E2)
                for s2 in range(2):
                    kt = 2 * t + s2
                    S = ps.tile([128, N], F32, name="S", tag="sq", bufs=2)
                    lhsT = KTs[p0 : p0 + 32, c, :, kt * 128 : (kt + 1) * 128]
                    for ch in range(2):
                        nc.tensor.matmul(
                            S[:, ch * 512 : (ch + 1) * 512],
                            lhsT,
                            QTs[p0 : p0 + 32, c, :, ch * 512 : (ch + 1) * 512],
                            start=True, stop=True,
                            perf_mode=DR,
                            tile_position=(p0, 0),
                        )
                    if s2 == 0:
                        # exact exp on ScalarE -> fp8
                        nc.scalar.activation(
                            out=E2[:, s2, :], in_=S, func=AF.Exp, scale=SCALE
                        )
                    else:
                        # Schraudolph trick on DVE -> uint8 bits == fp8 exp
                        nc.vector.tensor_scalar(
                            out=E2[:, s2, :].bitcast(U8), in0=S,
                            scalar1=A_TRICK * SCALE, scalar2=B_TRICK,
                            op0=ALU.mult, op1=ALU.add,
                        )
                if first and t == 0:
                    proj_v()
            for t in range(TB // 2):
                for ch in range(2):
                    nc.tensor.matmul(
                        O[:, ch * 512 : (ch + 1) * 512],
                        V2[:, 2 * t : 2 * t + 2, h, :],
                        E2s[t][:, :, ch * 512 : (ch + 1) * 512],
                        start=(t == 0), stop=(t == TB // 2 - 1),
                        perf_mode=DR,
                    )
            # Evacuate O (rows 0..64: O values + row 64 = softmax row sums)
            # so the PSUM accumulator frees fast; normalization happens later
            # from SBUF once the pair's reciprocals come back.
            Ocp = attn.tile([D + 1, N], F32, name="Ocp", tag="Ocp", bufs=4)
            nc.scalar.copy(out=Ocp, in_=O[0 : D + 1, :])
            return Ocp

        # Softmax-denominator pipeline, staged one head apart so no DVE
        # instruction ever parks at the queue head waiting on a DMA:
        #   coll(p)   at h=2p+1: tiny SBUF->SBUF DMAs of the two sum rows
        #   recip(p)  at h=2p+2: one [2,N] reciprocal + DRAM-bounce DMAs
        #   mults(p)  at h=2p+3: the two [64,N] normalizes into AO
        # The last pair collapses the stages with a PE broadcast (lower
        # latency than the DRAM bounce) since nothing runs after it.
        colls = {}
        r_bcs = {}

        def denom_collect(p, Ocp_a, Ocp_b):
            coll = attn.tile([2, N], F32, name="coll", tag="coll", bufs=2)
            nc.sync.dma_start(out=coll[0:1, :], in_=Ocp_a[D : D + 1, :])
            nc.sync.dma_start(out=coll[1:2, :], in_=Ocp_b[D : D + 1, :])
            colls[p] = coll

        def denom_recip(p, pe_broadcast):
            if pe_broadcast:
                # last pair: per-head [1,N] chains (ScalarE cannot read from
                # an unaligned partition base, so avoid [2,N] slicing here)
                for i, Ocp_x in enumerate((Ocps[2 * p], Ocps[2 * p + 1])):
                    r1 = attn.tile([1, N], F32, name="r1", tag="r1", bufs=2)
                    nc.vector.reciprocal(out=r1, in_=Ocp_x[D : D + 1, :])
                    r_bf = attn.tile([1, N], BF16, name="r_bf", tag="rbf", bufs=2)
                    nc.scalar.copy(out=r_bf, in_=r1)
                    bc = ps.tile([128, N], F32, name="bc", tag="sq", bufs=2)
                    for c0, c1 in ((0, 256), (256, 512), (512, 768), (768, N)):
                        nc.tensor.matmul(
                            bc[0:D, c0:c1],
                            ones_bf[0:1, 0:D],
                            r_bf[:, c0:c1],
                            start=True, stop=True,
                        )
                    r_bc = attn.tile([D, N], F32, name="r_bc", tag="rb", bufs=4)
                    nc.scalar.copy(out=r_bc, in_=bc[0:D, :])
                    r_bcs[(p, i)] = r_bc
            else:
                r2 = attn.tile([2, N], F32, name="r2", tag="r2", bufs=2)
                nc.vector.reciprocal(out=r2, in_=colls[p])
                nc.sync.dma_start(
                    out=_dram_ap(rsc_d, 2 * p * N, [[N, 2], [1, N]]), in_=r2
                )
                for i in range(2):
                    r_bc = attn.tile([D, N], F32, name="r_bc", tag="rb", bufs=4)
                    nc.sync.dma_start(
                        out=r_bc,
                        in_=_dram_ap(rsc_d, (2 * p + i) * N, [[0, D], [1, N]]),
                    )
                    r_bcs[(p, i)] = r_bc

        def denom_mults(p, split=False):
            for i in range(2):
                hx = 2 * p + i
                kbh, ro = hx // 2, D * (hx % 2)
                Ocp_x = Ocps[hx]
                cols = ((0, 512), (512, N)) if split else ((0, N),)
                for c0, c1 in cols:
                    nc.vector.tensor_mul(
                        out=AO[ro : ro + D, kbh, c0:c1],
                        in0=Ocp_x[0:D, c0:c1], in1=r_bcs[(p, i)][:, c0:c1],
                    )

        # ---- stage 1+2+3 interleaved: projections feed attention --------
        proj_qk(0, first=True)
        Ocps = {}
        for h in range(H):
            Ocps[h] = attend(h, first=(h == 0))
            if h % 4 == 0 and h // 4 < NCH - 1:
                proj_qk(h // 4 + 1, first=False)
            if h % 2 == 1 and h != H - 1:
                denom_collect((h - 1) // 2, Ocps[h - 1], Ocps[h])
            if h % 2 == 0 and h >= 2:
                denom_recip((h - 2) // 2, pe_broadcast=False)
            if h % 2 == 1 and h >= 3:
                denom_mults((h - 3) // 2)
        # tail: pair 5 (pairs 0-4 completed inside the loop)
        denom_recip(5, pe_broadcast=True)
        denom_mults(5, split=True)

        # ---- stage 4: out-proj + residual + LayerNorm -------------------
        # Mean comes from WoT's extra row-mean column plus the host-computed
        # residual mean; sum-of-squares from DVE scalar_tensor_tensor with
        # accum_out (keeps ScalarE free for the exp stream's tail).
        ypools = [
            lambda: psP.tile([128, N], F32, name="Y", tag="P", bufs=1),
            lambda: ps.tile([128, N], F32, name="Y", tag="sq", bufs=2),
            lambda: psO.tile([128, N], F32, name="Y", tag="O", bufs=1),
        ]
        for tb in range(TB):
            Y = ypools[tb % 3]()
            for fbp in range(KB // 2):
                lhsT = AO[:, 2 * fbp : 2 * fbp + 2, tb * 128 : (tb + 1) * 128]
                for c0, c1 in ((0, 512), (512, CO)):
                    nc.tensor.matmul(
                        Y[:, c0:c1], lhsT,
                        WoT[:, 2 * fbp : 2 * fbp + 2, c0:c1],
                        start=(fbp == 0), stop=False,
                        perf_mode=DR,
                    )
            # residual add on the (idle) PE: identity-weight matmul
            # accumulating bf16 q_res into the same PSUM group
            for c0, c1 in ((0, 512), (512, CO)):
                nc.tensor.matmul(
                    Y[:, c0:c1], ident,
                    q_sb[:, tb, c0:c1],
                    start=False, stop=True,
                )
            x1 = Y[:, 0:C]
            # mean/variance in one pass: two bn_stats halves + bn_aggr
            # (walrus only allows ONE PSUM operand per DVE instruction, so
            # no x*x tensor_tensor here)
            bst = epi.tile([128, 2, 6], F32, name="bst", tag="bst", bufs=4)
            nc.vector.bn_stats(out=bst[:, 0, :], in_=x1[:, 0 : C // 2])
            nc.vector.bn_stats(out=bst[:, 1, :], in_=x1[:, C // 2 : C])
            mv = epi.tile([128, 2], F32, name="mv", tag="mv", bufs=6)
            nc.vector.bn_aggr(out=mv, in_=bst)
            mu = mv[:, 0:1]
            sd = epi.tile([128, 1], F32, name="sd", tag="sd", bufs=6)
            nc.scalar.activation(
                out=sd, in_=mv[:, 1:2], func=AF.Sqrt,
                bias=eps_t[:, 0:1], scale=1.0,
            )
            rs = epi.tile([128, 1], F32, name="rs", tag="rs", bufs=6)
            nc.vector.reciprocal(out=rs, in_=sd)
            if tb % 2 == 0 or tb == TB - 1:
                nmr = epi.tile([128, 1], F32, name="nmr", tag="nmr", bufs=6)
                nc.vector.tensor_scalar(
                    out=nmr, in0=mu, scalar1=rs, scalar2=-1.0,
                    op0=ALU.mult, op1=ALU.mult,
                )
            xn = epi.tile([128, C], BF16, name="xn", tag="xn", bufs=5)
            if tb == TB - 1:
                # final tile: compute the two halves on ScalarE and DVE in
                # parallel and ship two half-DMAs so the last transfer starts
                # as early as possible
                nc.scalar.activation(
                    out=xn[:, 0 : C // 2], in_=x1[:, 0 : C // 2],
                    func=AF.Identity, bias=nmr, scale=rs,
                )
                nc.vector.tensor_scalar(
                    out=xn[:, C // 2 : C], in0=x1[:, C // 2 : C],
                    scalar1=mu, scalar2=rs, op0=ALU.subtract, op1=ALU.mult,
                )
                for hh in range(2):
                    nc.sync.dma_start(
                        out=_dram_ap(
                            out_t, tb * 128 * C + hh * (C // 2),
                            [[C, 128], [1, C // 2]],
                        ),
                        in_=xn[:, hh * (C // 2) : (hh + 1) * (C // 2)],
                    )
                return
            if tb % 2 == 0:
                nc.scalar.activation(
                    out=xn, in_=x1, func=AF.Identity, bias=nmr, scale=rs,
                )
            else:
                nc.vector.tensor_scalar(
                    out=xn, in0=x1, scalar1=mu, scalar2=rs,
                    op0=ALU.subtract, op1=ALU.mult,
                )
            nc.sync.dma_start(
                out=_dram_ap(out_t, tb * 128 * C, [[C, 128], [1, C]]),
                in_=xn,
            )


# ---------------------------------------------------------------------------
# Entry point
# ---------------------------------------------------------------------------
_nc_cache = None


def _get_nc():
    global _nc_cache
    if _nc_cache is None:
        _install_compile_hook()
        _nc_cache = build_nc()
    return _nc_cache


def _qk_perm() -> np.ndarray:
    """Column gather for Wq/Wk: output block j=2c+s (128 wide), partition
    p holds head 4c+p//32, d=(p%32)+32s (DoubleRow S layout, head bases
    0/32/64/96)."""
    perm = np.zeros(CP, np.int64)
    for c in range(NCH):
        for s in range(2):
            j = 2 * c + s
            p = np.arange(128)
            perm[j * 128 + p] = (4 * c + p // 32) * 64 + (p % 32) + 32 * s
    return perm


def make_in_maps(inputs: dict) -> list:
    """Host-side marshaling: shard over batch, pre-transpose to feature-major,
    pre-quantize matmul operands to fp8e4, permute Wq/Wk output features for
    the DoubleRow S layout, fold the out-proj bias into the residual."""
    arrs = {k: np.asarray(v, dtype=np.float32) for k, v in inputs.items()}
    perm = _qk_perm()
    WqTp = arrs["Wq"].T[:, perm].astype(F8_NP)
    WkTp = arrs["Wk"].T[:, perm].astype(F8_NP)
    WoT = arrs["Wo"].T
    WoT_aug = np.concatenate([WoT, WoT.mean(axis=1, keepdims=True)], axis=1)

    def bias_pack(b):
        return b[perm].reshape(NJ, 128)

    bqk_flat = np.concatenate(
        [bias_pack(arrs["bq"]), bias_pack(arrs["bk"])]
    )  # [2*NJ, 128]
    shared = {
        "WvT8": np.ascontiguousarray(arrs["Wv"].T.astype(F8_NP)),
        "WoT8": np.ascontiguousarray(WoT_aug.astype(F8_NP)),
        "bqk": np.ascontiguousarray(bqk_flat.T),  # [128, 2*NJ]
        "bvT": np.ascontiguousarray(
            arrs["bv"].reshape(1, C).astype(ml_dtypes.bfloat16)
        ),
    }
    in_maps = []
    for b in range(B):
        m = dict(shared)
        m["QW8"] = np.ascontiguousarray(
            np.concatenate([WqTp, arrs["query"][b].T.astype(F8_NP)], axis=1)
        )
        m["KW8"] = np.ascontiguousarray(
            np.concatenate([WkTp, arrs["context"][b].T.astype(F8_NP)], axis=1)
        )
        q_res = (arrs["query"][b] + arrs["bo"]).astype(ml_dtypes.bfloat16)
        m["q_res"] = np.ascontiguousarray(
            np.concatenate(
                [q_res, np.zeros((N, 1), ml_dtypes.bfloat16)], axis=1
            )
        )
        m["ident"] = np.eye(128, dtype=ml_dtypes.bfloat16)
        in_maps.append(m)
    return in_maps


def kernel(**inputs) -> np.ndarray:
    from concourse.bass_utils import run_bass_kernel_spmd

    nc = _get_nc()
    in_maps = make_in_maps(inputs)
    res = run_bass_kernel_spmd(nc, in_maps, core_ids=list(range(B)))
    out = np.stack(
        [np.asarray(r["out"], dtype=np.float32) for r in res.results]
    )
    # LayerNorm's final affine is applied host-side (it commutes out of the
    # kernel: out = xn * gamma + beta).
    gamma = np.asarray(inputs["ln_gamma"], np.float32)
    beta = np.asarray(inputs["ln_beta"], np.float32)
    return out * gamma + beta


# revision 12
# speedup vs baseline: 1.0278x; 1.0278x over previous
"""Trainium2 Bass kernel for nn_CrossAttentionBlock (B=8, N=1024, C=768, H=12).

Sharding: data-parallel over the batch dim - each of the 8 NeuronCores runs the
full cross-attention block for one batch element. No collectives.

v2 vs v1: the Activation engine (softmax exp) was the bottleneck at 85% busy.
Rebalanced by computing roughly half the exp tiles on DVE via a Schraudolph
bit-trick (uint8(A*S + B) bitcast to fp8e4m3 approximates exp(S/8) to ~3% RMS,
which is at the same level as the fp8 quantization the E tiles already get),
moving all PSUM evacuations to whichever of ACT/DVE balances, widening the
Q/K projection blocks to 128 partitions (4 heads x 32, tile_position=(96,0)
for the 4th head), folding the V bias into a bf16 rank-1 PE matmul so the V
evacuation is a pure copy, batching the softmax-denominator reciprocals in
head pairs via tiny SBUF->SBUF collector DMAs, and computing the LayerNorm
sum-of-squares on DVE (scalar_tensor_tensor with accum_out).
"""

import json

import ml_dtypes
import numpy as np

import concourse.bass as bass
import concourse.mybir as mybir
import concourse.tile as tile

B, N, C, H, D = 8, 1024, 768, 12, 64
KB = C // 128  # feature-dim 128-blocks
TB = N // 128  # token-dim 128-blocks
NCH = 3        # head chunks of 4 heads (partition bases 0/32/64/96)
NJ = 2 * NCH   # 128-wide packed Q/K output blocks per tensor
CP = NJ * 128  # packed Q/K out-feature count (768)
SCALE = D ** -0.5
EPS = 1e-5
F32 = mybir.dt.float32
BF16 = mybir.dt.bfloat16
FP8 = mybir.dt.float8e4
U8 = mybir.dt.uint8
AF = mybir.ActivationFunctionType
ALU = mybir.AluOpType
DR = mybir.MatmulPerfMode.DoubleRow
F8_NP = ml_dtypes.float8_e4m3

# Schraudolph exp: uint8(A*x + B) bitcast fp8e4m3 ~= exp(x), RMS err ~3.1%.
# B tuned for zero mean relative error so mixed exact/trick softmax rows
# stay consistent.  Range check: S*SCALE in [-2.9, 2.9] -> byte in [22, 89].
A_TRICK = 8.0 / np.log(2.0)
B_TRICK = 56.0 - 0.45

# ---------------------------------------------------------------------------
# Workaround: this walrus build rejects instructions with more than one
# semaphore wait ("Too many sync wait commands").  Legalize the BIR by hoisting
# excess waits onto same-engine NoOps inserted right before the instruction.
# ---------------------------------------------------------------------------
_MAX_WAITS = 1
_legal_counter = [0]


def _legalize_waits(bir_json: bytes) -> bytes:
    m = json.loads(bir_json)
    changed = False
    for fn in m.get("functions", []):
        for bb in fn.get("blocks", []):
            out = []
            for inst in bb.get("instructions", []):
                si = inst.get("sync_info") or {}
                waits = si.get("on_wait") or []
                if len(waits) > _MAX_WAITS:
                    changed = True
                    extra = waits[_MAX_WAITS:]
                    si["on_wait"] = waits[:_MAX_WAITS]
                    for i in range(0, len(extra), _MAX_WAITS):
                        _legal_counter[0] += 1
                        nop = {
                            "engine": inst["engine"],
                            "ins": [],
                            "name": f"I-legalw-{_legal_counter[0]}",
                            "opcode": "NoOp",
                            "outs": [],
                            "sync_info": {
                                "on_update": [],
                                "on_wait": extra[i : i + _MAX_WAITS],
                            },
                        }
                        if "debug" in inst:
                            nop["debug"] = inst["debug"]
                        out.append(nop)
                out.append(inst)
            bb["instructions"] = out
    return json.dumps(m).encode() if changed else bir_json


_hooked = False


def _install_compile_hook():
    global _hooked
    if _hooked:
        return
    _hooked = True
    import concourse.bass_utils as bu

    orig = bu.compile_bir_kernel

    def compile_bir_kernel(bir_json, tmpdir, neff_name="file.neff"):
        return orig(_legalize_waits(bir_json), tmpdir, neff_name)

    bu.compile_bir_kernel = compile_bir_kernel
    try:
        import concourse.bass2jax as b2j

        b2j.compile_bir_kernel = compile_bir_kernel
    except ImportError:
        pass


# ---------------------------------------------------------------------------
# Kernel builder
# ---------------------------------------------------------------------------

def _dram_ap(t, offset, ap):
    return bass.AP(t, offset, ap)


def build_nc() -> bass.Bass:
    nc = bass.Bass()

    QW_d = nc.dram_tensor("QW8", [C, CP + N], FP8, kind="ExternalInput")
    KW_d = nc.dram_tensor("KW8", [C, CP + N], FP8, kind="ExternalInput")
    WvT_d = nc.dram_tensor("WvT8", [C, C], FP8, kind="ExternalInput")
    WoT_d = nc.dram_tensor("WoT8", [C, C + 1], FP8, kind="ExternalInput")
    bqk = nc.dram_tensor("bqk", [128, 2 * NJ], F32, kind="ExternalInput")
    bvT = nc.dram_tensor("bvT", [1, C], BF16, kind="ExternalInput")
    qres_d = nc.dram_tensor("q_res", [N, C + 1], BF16, kind="ExternalInput")
    ident_d = nc.dram_tensor("ident", [128, 128], BF16, kind="ExternalInput")
    out_t = nc.dram_tensor("out", [N, C], BF16, kind="ExternalOutput")
    rsc_d = nc.dram_tensor("r_scratch", [H, N], F32, kind="Internal")

    with tile.TileContext(nc) as tc:
        _body(tc, nc, (QW_d, KW_d, WvT_d, WoT_d),
              (bqk, bvT), qres_d, ident_d, out_t, rsc_d)
    return nc


def _body(tc, nc, WTs, bs, qres_d, ident_d, out_t, rsc_d):
    QW_d, KW_d, WvT_d, WoT_d = WTs
    bqk, bvT = bs

    with (
        tc.tile_pool(name="singles", bufs=1) as singles,
        tc.tile_pool(name="feat", bufs=1) as feat,
        tc.tile_pool(name="attn", bufs=1) as attn,
        tc.tile_pool(name="epi", bufs=1) as epi,
        tc.tile_pool(name="ps", bufs=1, space="PSUM") as ps,
        tc.tile_pool(name="psO", bufs=1, space="PSUM") as psO,
    ):
        # ---- constants / biases (tiny, issued first) --------------------
        bqk_sb = singles.tile([128, 2, NJ], F32, name="bqk_sb")
        nc.sync.dma_start(
            out=bqk_sb, in_=_dram_ap(bqk, 0, [[2 * NJ, 128], [1, 2 * NJ]])
        )
        bq_sb = bqk_sb[:, 0, :]
        bk_sb = bqk_sb[:, 1, :]
        eps_t = singles.tile([128, 1], F32, name="eps_t")
        nc.vector.memset(eps_t, EPS)
        warm = singles.tile([1, 512], FP8, name="warm")
        nc.vector.memset(warm, 0.0)
        ones_bf = singles.tile([1, 128], BF16, name="ones_bf")
        nc.vector.memset(ones_bf, 1.0)
        bvT_sb = singles.tile([1, C], BF16, name="bvT_sb")
        nc.sync.dma_start(out=bvT_sb, in_=_dram_ap(bvT, 0, [[C, 1], [1, C]]))
        pwarm = psO.tile([128, N], F32, name="pwarm", tag="O", bufs=1)
        for _ in range(8):
            nc.tensor.matmul(
                pwarm[0:1, 0:512], warm[0:1, 0:1], warm, start=True, stop=True
            )

        # ---- long-lived fp8 feature-major tensors -----------------------
        CPN = CP + N
        QW = feat.tile([128, KB, CPN], FP8, name="QW")
        KW = feat.tile([128, KB, CPN], FP8, name="KW")
        for t_sb, t_d in ((QW, QW_d), (KW, KW_d)):
            for k0, k1 in ((0, 4), (4, KB)):
                nc.sync.dma_start(
                    out=t_sb[:, k0:k1, :],
                    in_=_dram_ap(
                        t_d, k0 * 128 * CPN,
                        [[CPN, 128], [128 * CPN, k1 - k0], [1, CPN]],
                    ),
                )
        WqT = QW[:, :, 0:CP]
        qT = QW[:, :, CP:CPN]
        WkT = KW[:, :, 0:CP]
        cT = KW[:, :, CP:CPN]
        WvT = feat.tile([128, KB, C], FP8, name="WvT")
        nc.sync.dma_start(
            out=WvT, in_=_dram_ap(WvT_d, 0, [[C, 128], [128 * C, KB], [1, C]])
        )
        CO = C + 1
        WoT = feat.tile([128, KB, CO], FP8, name="WoT")
        nc.sync.dma_start(
            out=WoT, in_=_dram_ap(WoT_d, 0, [[CO, 128], [128 * CO, KB], [1, CO]])
        )
        CQ = C + 1
        q_sb = feat.tile([128, TB, CQ], BF16, name="q_sb")
        nc.sync.dma_start(
            out=q_sb, in_=_dram_ap(qres_d, 0, [[CQ, 128], [128 * CQ, TB], [1, CQ]])
        )
        ident = feat.tile([128, 128], BF16, name="ident")
        nc.sync.dma_start(out=ident, in_=_dram_ap(ident_d, 0, [[128, 128], [1, 128]]))

        # Q/K packed layout [128, chunk, sub, N]: block j=2c+s holds, at
        # partition p, head 4c+p//32, feature d=(p%32)+32s.
        QTs = feat.tile([128, NCH, 2, N], FP8, name="QTs")
        KTs = feat.tile([128, NCH, 2, N], FP8, name="KTs")
        # V token-major, one 128-wide slot per head: cols 0-63 = V, col 64 =
        # ones (softmax row sums ride along in the O matmul), 65+ = zeros.
        V2 = feat.tile([128, TB, H, 128], FP8, name="V2")
        nc.gpsimd.memset(V2[:, :, :, D : D + 1], 1.0)
        nc.gpsimd.memset(V2[:, :, :, D + 1 : 128], 0.0)
        AO = feat.tile([128, KB, N], FP8, name="AO")

        def proj_qk(c, first):
            """Q/K projection blocks j=2c,2c+1 (128 partitions = 4 heads),
            bias-add fused into the fp8 evacuation."""
            for ti, (wT, srcT, b_sb, dstT) in enumerate((
                (WqT, qT, bq_sb, QTs),
                (WkT, cT, bk_sb, KTs),
            )):
                for s in range(2):
                    j = 2 * c + s
                    pj = ps.tile([128, N], F32, name="pj", tag="sq", bufs=3)
                    for kbp in range(KB // 2):
                        lhsT = wT[:, 2 * kbp : 2 * kbp + 2,
                                  j * 128 : (j + 1) * 128]
                        for ch in range(2):
                            nc.tensor.matmul(
                                pj[:, ch * 512 : (ch + 1) * 512],
                                lhsT,
                                srcT[:, 2 * kbp : 2 * kbp + 2,
                                     ch * 512 : (ch + 1) * 512],
                                start=(kbp == 0),
                                stop=(kbp == KB // 2 - 1),
                                perf_mode=DR,
                            )
                    # evacuation engine: alternate ACT/DVE
                    on_act = (ti == 0)
                    splits = (0, 256, N) if (first and ti == 1 and s == 0) \
                        else (0, N)
                    for c0, c1 in zip(splits, splits[1:]):
                        if on_act:
                            nc.scalar.activation(
                                out=dstT[:, c, s, c0:c1],
                                in_=pj[:, c0:c1],
                                func=AF.Identity,
                                bias=b_sb[:, j : j + 1], scale=1.0,
                            )
                        else:
                            nc.vector.tensor_scalar(
                                out=dstT[:, c, s, c0:c1],
                                in0=pj[:, c0:c1],
                                scalar1=b_sb[:, j : j + 1], scalar2=None,
                                op0=ALU.add,
                            )

        def proj_v():
            for tb in range(TB):
                pvt = ps.tile([128, N], F32, name="pv", tag="sq", bufs=3)
                pv = pvt[:, 0:C]
                for kbp in range(KB // 2):
                    lhsT = cT[:, 2 * kbp : 2 * kbp + 2, tb * 128 : (tb + 1) * 128]
                    for c0, c1 in ((0, 512), (512, C)):
                        nc.tensor.matmul(
                            pv[:, c0:c1], lhsT,
                            WvT[:, 2 * kbp : 2 * kbp + 2, c0:c1],
                            start=(kbp == 0), stop=False,
                            perf_mode=DR,
                        )
                # bias: rank-1 bf16 matmul ones^T x bvT accumulated into the
                # same PSUM group, so the evacuation is a pure quantize copy
                # (engine-flexible).
                for c0, c1 in ((0, 512), (512, C)):
                    nc.tensor.matmul(
                        pv[:, c0:c1],
                        ones_bf[0:1, 0:128],
                        bvT_sb[0:1, c0:c1],
                        start=False, stop=True,
                    )
                if tb % 2 == 0:
                    nc.scalar.copy(
                        out=V2[:, tb, :, 0:D],
                        in_=pv.rearrange("p (h d) -> p h d", h=H),
                    )
                else:
                    nc.vector.tensor_copy(
                        out=V2[:, tb, :, 0:D],
                        in_=pv.rearrange("p (h d) -> p h d", h=H),
                    )

        def attend(h, first):
            c, hp = h // 4, h % 4
            p0 = hp * 32
            kbh, ro = h // 2, D * (h % 2)
            # All 8 S matmuls (and their exps) are issued BEFORE the O
            # matmuls (program order on PE keeps the S stream fed).
            O = psO.tile([128, N], F32, name="O", tag="O", bufs=1)
            E2s = []
            for t in range(TB // 2):
                E2 = attn.tile([128, 2, N], FP8, name="E2", tag="E2", bufs=8)
                E2s.append(E2)
                for s2 in range(2):
                    kt = 2 * t + s2
                    S = ps.tile([128, N], F32, name="S", tag="sq", bufs=3)
                    lhsT = KTs[p0 : p0 + 32, c, :, kt * 128 : (kt + 1) * 128]
                    for ch in range(2):
                        nc.tensor.matmul(
                            S[:, ch * 512 : (ch + 1) * 512],
                            lhsT,
                            QTs[p0 : p0 + 32, c, :, ch * 512 : (ch + 1) * 512],
                            start=True, stop=True,
                            perf_mode=DR,
                            tile_position=(p0, 0),
                        )
                    if s2 == 0:
                        # exact exp on ScalarE -> fp8
                        nc.scalar.activation(
                            out=E2[:, s2, :], in_=S, func=AF.Exp, scale=SCALE
                        )
                    else:
                        # Schraudolph trick on DVE -> uint8 bits == fp8 exp
                        nc.vector.tensor_scalar(
                            out=E2[:, s2, :].bitcast(U8), in0=S,
                            scalar1=A_TRICK * SCALE, scalar2=B_TRICK,
                            op0=ALU.mult, op1=ALU.add,
                        )
                if first and t == 0:
                    proj_v()
            for t in range(TB // 2):
                for ch in range(2):
                    nc.tensor.matmul(
                        O[:, ch * 512 : (ch + 1) * 512],
                        V2[:, 2 * t : 2 * t + 2, h, :],
                        E2s[t][:, :, ch * 512 : (ch + 1) * 512],
                        start=(t == 0), stop=(t == TB // 2 - 1),
                        perf_mode=DR,
                    )
            # Evacuate O (rows 0..64: O values + row 64 = softmax row sums)
            # so the PSUM accumulator frees fast; normalization happens later
            # from SBUF once the pair's reciprocals come back.
            if h == H - 1:
                return O
            Ocp = attn.tile([D + 1, N], F32, name="Ocp", tag="Ocp", bufs=4)
            nc.scalar.copy(out=Ocp, in_=O[0 : D + 1, :])
            return Ocp

        # Softmax-denominator pipeline, staged one head apart so no DVE
        # instruction ever parks at the queue head waiting on a DMA:
        #   coll(p)   at h=2p+1: tiny SBUF->SBUF DMAs of the two sum rows
        #   recip(p)  at h=2p+2: one [2,N] reciprocal + DRAM-bounce DMAs
        #   mults(p)  at h=2p+3: the two [64,N] normalizes into AO
        # The last pair collapses the stages with a PE broadcast (lower
        # latency than the DRAM bounce) since nothing runs after it.
        colls = {}
        r_bcs = {}

        def denom_collect(p, Ocp_a, Ocp_b):
            coll = attn.tile([2, N], F32, name="coll", tag="coll", bufs=2)
            nc.sync.dma_start(out=coll[0:1, :], in_=Ocp_a[D : D + 1, :])
            nc.sync.dma_start(out=coll[1:2, :], in_=Ocp_b[D : D + 1, :])
            colls[p] = coll

        def denom_recip(p):
            r2 = attn.tile([2, N], F32, name="r2", tag="r2", bufs=2)
            nc.vector.reciprocal(out=r2, in_=colls[p])
            nc.sync.dma_start(
                out=_dram_ap(rsc_d, 2 * p * N, [[N, 2], [1, N]]), in_=r2
            )
            for i in range(2):
                r_bc = attn.tile([D, N], F32, name="r_bc", tag="rb", bufs=4)
                nc.sync.dma_start(
                    out=r_bc,
                    in_=_dram_ap(rsc_d, (2 * p + i) * N, [[0, D], [1, N]]),
                )
                r_bcs[(p, i)] = r_bc

        def fast_norm(hx, Osrc, nchunk):
            """Tail path for the last two heads: reciprocal straight off the
            sums row, PE broadcast, normalize in column chunks so the
            out-projection can start on early token blocks ASAP."""
            r1 = attn.tile([1, N], F32, name="r1", tag="r1", bufs=2)
            nc.vector.reciprocal(out=r1, in_=Osrc[D : D + 1, :])
            r_bf = attn.tile([1, N], BF16, name="r_bf", tag="rbf", bufs=2)
            nc.scalar.copy(out=r_bf, in_=r1)
            bc = ps.tile([128, N], F32, name="bc", tag="sq", bufs=3)
            for c0, c1 in ((0, 256), (256, 512), (512, 768), (768, N)):
                nc.tensor.matmul(
                    bc[0:D, c0:c1],
                    ones_bf[0:1, 0:D],
                    r_bf[:, c0:c1],
                    start=True, stop=True,
                )
            r_bc = attn.tile([D, N], F32, name="r_bc", tag="rb", bufs=4)
            nc.scalar.copy(out=r_bc, in_=bc[0:D, :])
            kbh, ro = hx // 2, D * (hx % 2)
            cw = N // nchunk
            for ci in range(nchunk):
                c0, c1 = ci * cw, (ci + 1) * cw
                nc.vector.tensor_mul(
                    out=AO[ro : ro + D, kbh, c0:c1],
                    in0=Osrc[0:D, c0:c1], in1=r_bc[:, c0:c1],
                )

        def denom_mults(p):
            for i in range(2):
                hx = 2 * p + i
                kbh, ro = hx // 2, D * (hx % 2)
                nc.vector.tensor_mul(
                    out=AO[ro : ro + D, kbh, :],
                    in0=Ocps[hx][0:D, :], in1=r_bcs[(p, i)],
                )

        # ---- stage 1+2+3 interleaved: projections feed attention --------
        proj_qk(0, first=True)
        Ocps = {}
        for h in range(H):
            Ocps[h] = attend(h, first=(h == 0))
            if h % 4 == 0 and h // 4 < NCH - 1:
                proj_qk(h // 4 + 1, first=False)
            if h % 2 == 1 and h < H - 2:
                denom_collect((h - 1) // 2, Ocps[h - 1], Ocps[h])
            if h % 2 == 0 and h >= 2:
                denom_recip((h - 2) // 2)
            if h % 2 == 1 and h >= 3 and h < H - 1:
                denom_mults((h - 3) // 2)
            if h == H - 2:
                fast_norm(h, Ocps[h], nchunk=2)
        # tail: head 11 normalizes straight from PSUM in 256-col chunks
        denom_mults(4)
        fast_norm(H - 1, Ocps[H - 1], nchunk=4)

        # ---- stage 4: out-proj + residual + LayerNorm -------------------
        # Mean comes from WoT's extra row-mean column plus the host-computed
        # residual mean; sum-of-squares from DVE scalar_tensor_tensor with
        # accum_out (keeps ScalarE free for the exp stream's tail).
        ypools = [
            lambda: ps.tile([128, N], F32, name="Y", tag="sq", bufs=3),
            lambda: ps.tile([128, N], F32, name="Y", tag="sq", bufs=3),
            lambda: psO.tile([128, N], F32, name="Y", tag="O", bufs=1),
            lambda: ps.tile([128, N], F32, name="Y", tag="sq", bufs=3),
        ]
        for tb in range(TB):
            Y = ypools[tb % 4]()
            for fbp in range(KB // 2):
                lhsT = AO[:, 2 * fbp : 2 * fbp + 2, tb * 128 : (tb + 1) * 128]
                for c0, c1 in ((0, 512), (512, CO)):
                    nc.tensor.matmul(
                        Y[:, c0:c1], lhsT,
                        WoT[:, 2 * fbp : 2 * fbp + 2, c0:c1],
                        start=(fbp == 0), stop=False,
                        perf_mode=DR,
                    )
            # residual add on the (idle) PE: identity-weight matmul
            # accumulating bf16 q_res into the same PSUM group
            for c0, c1 in ((0, 512), (512, CO)):
                nc.tensor.matmul(
                    Y[:, c0:c1], ident,
                    q_sb[:, tb, c0:c1],
                    start=False, stop=True,
                )
            x1 = Y[:, 0:C]
            # mean/variance in one pass: two bn_stats halves + bn_aggr
            # (walrus only allows ONE PSUM operand per DVE instruction, so
            # no x*x tensor_tensor here)
            bst = epi.tile([128, 2, 6], F32, name="bst", tag="bst", bufs=4)
            nc.vector.bn_stats(out=bst[:, 0, :], in_=x1[:, 0 : C // 2])
            nc.vector.bn_stats(out=bst[:, 1, :], in_=x1[:, C // 2 : C])
            mv = epi.tile([128, 2], F32, name="mv", tag="mv", bufs=6)
            nc.vector.bn_aggr(out=mv, in_=bst)
            mu = mv[:, 0:1]
            sd = epi.tile([128, 1], F32, name="sd", tag="sd", bufs=6)
            nc.scalar.activation(
                out=sd, in_=mv[:, 1:2], func=AF.Sqrt,
                bias=eps_t[:, 0:1], scale=1.0,
            )
            rs = epi.tile([128, 1], F32, name="rs", tag="rs", bufs=6)
            nc.vector.reciprocal(out=rs, in_=sd)
            nmr = epi.tile([128, 1], F32, name="nmr", tag="nmr", bufs=6)
            nc.vector.tensor_scalar(
                out=nmr, in0=mu, scalar1=rs, scalar2=-1.0,
                op0=ALU.mult, op1=ALU.mult,
            )
            xn = epi.tile([128, C], BF16, name="xn", tag="xn", bufs=5)
            if tb == TB - 1:
                # final tile: compute the two halves on ScalarE and DVE in
                # parallel and ship two half-DMAs so the last transfer starts
                # as early as possible
                nc.scalar.activation(
                    out=xn[:, 0 : C // 2], in_=x1[:, 0 : C // 2],
                    func=AF.Identity, bias=nmr, scale=rs,
                )
                nc.vector.tensor_scalar(
                    out=xn[:, C // 2 : C], in0=x1[:, C // 2 : C],
                    scalar1=mu, scalar2=rs, op0=ALU.subtract, op1=ALU.mult,
                )
                for hh in range(2):
                    nc.sync.dma_start(
                        out=_dram_ap(
                            out_t, tb * 128 * C + hh * (C // 2),
                            [[C, 128], [1, C // 2]],
                        ),
                        in_=xn[:, hh * (C // 2) : (hh + 1) * (C // 2)],
                    )
                return
            nc.scalar.activation(
                out=xn, in_=x1, func=AF.Identity, bias=nmr, scale=rs,
            )
            nc.sync.dma_start(
                out=_dram_ap(out_t, tb * 128 * C, [[C, 128], [1, C]]),
                in_=xn,
            )


# ---------------------------------------------------------------------------
# Entry point
# ---------------------------------------------------------------------------
_nc_cache = None


def _get_nc():
    global _nc_cache
    if _nc_cache is None:
        _install_compile_hook()
        _nc_cache = build_nc()
    return _nc_cache


def _qk_perm() -> np.ndarray:
    """Column gather for Wq/Wk: output block j=2c+s (128 wide), partition
    p holds head 4c+p//32, d=(p%32)+32s (DoubleRow S layout, head bases
    0/32/64/96)."""
    perm = np.zeros(CP, np.int64)
    for c in range(NCH):
        for s in range(2):
            j = 2 * c + s
            p = np.arange(128)
            perm[j * 128 + p] = (4 * c + p // 32) * 64 + (p % 32) + 32 * s
    return perm


def make_in_maps(inputs: dict) -> list:
    """Host-side marshaling: shard over batch, pre-transpose to feature-major,
    pre-quantize matmul operands to fp8e4, permute Wq/Wk output features for
    the DoubleRow S layout, fold the out-proj bias into the residual."""
    arrs = {k: np.asarray(v, dtype=np.float32) for k, v in inputs.items()}
    perm = _qk_perm()
    WqTp = arrs["Wq"].T[:, perm].astype(F8_NP)
    WkTp = arrs["Wk"].T[:, perm].astype(F8_NP)
    WoT = arrs["Wo"].T
    WoT_aug = np.concatenate([WoT, WoT.mean(axis=1, keepdims=True)], axis=1)

    def bias_pack(b):
        return b[perm].reshape(NJ, 128)

    bqk_flat = np.concatenate(
        [bias_pack(arrs["bq"]), bias_pack(arrs["bk"])]
    )  # [2*NJ, 128]
    shared = {
        "WvT8": np.ascontiguousarray(arrs["Wv"].T.astype(F8_NP)),
        "WoT8": np.ascontiguousarray(WoT_aug.astype(F8_NP)),
        "bqk": np.ascontiguousarray(bqk_flat.T),  # [128, 2*NJ]
        "bvT": np.ascontiguousarray(
            arrs["bv"].reshape(1, C).astype(ml_dtypes.bfloat16)
        ),
    }
    in_maps = []
    for b in range(B):
        m = dict(shared)
        m["QW8"] = np.ascontiguousarray(
            np.concatenate([WqTp, arrs["query"][b].T.astype(F8_NP)], axis=1)
        )
        m["KW8"] = np.ascontiguousarray(
            np.concatenate([WkTp, arrs["context"][b].T.astype(F8_NP)], axis=1)
        )
        q_res = (arrs["query"][b] + arrs["bo"]).astype(ml_dtypes.bfloat16)
        m["q_res"] = np.ascontiguousarray(
            np.concatenate(
                [q_res, np.zeros((N, 1), ml_dtypes.bfloat16)], axis=1
            )
        )
        m["ident"] = np.eye(128, dtype=ml_dtypes.bfloat16)
        in_maps.append(m)
    return in_maps


def kernel(**inputs) -> np.ndarray:
    from concourse.bass_utils import run_bass_kernel_spmd

    nc = _get_nc()
    in_maps = make_in_maps(inputs)
    res = run_bass_kernel_spmd(nc, in_maps, core_ids=list(range(B)))
    out = np.stack(
        [np.asarray(r["out"], dtype=np.float32) for r in res.results]
    )
    # LayerNorm's final affine is applied host-side (it commutes out of the
    # kernel: out = xn * gamma + beta).
    gamma = np.asarray(inputs["ln_gamma"], np.float32)
    beta = np.asarray(inputs["ln_beta"], np.float32)
    return out * gamma + beta
